# revision 49
# baseline (speedup 1.0000x reference)
"""Gaussian-kernel attention for Trainium2 (Bass/Tile), 8-core data-parallel.

Computes out = x + K @ x with K = exp(-r * d2), d2[t,s] = ||x_t - x_s||^2,
per batch.  Decomposition used on-chip:

    d2 = sq_t + sq_s - 2*G          (G = X X^T, sq = rowwise |x|^2)
    K  = e_t * exp(2r*G) * e_s      (e_i = exp(-r*sq_i))
    out[t] = x[t] + e_t * sum_s exp(2r*G)[s,t] * (e_s * x[s])

Performance architecture (all-bf16 matmuls; fp8 was tried and rejected —
its quantization noise alone exceeds the 2e-2 error budget):

  * mm1 (G = X X^T, K=64 contraction) runs as CONCURRENT dual row-tile
    pairs: two s-blocks issue back-to-back into PE row groups at
    tile_position (0,0) and (64,0); the duplicated x^T layout (xt) feeds
    both halves, so a pair of 512-col matmuls spans ~one matmul time.
    mm1 runs TWO steps ahead of mm2 (g2 triple-buffered) so the exp
    latency never blocks the PE's strict-FIFO queue head — PE-idle
    micro-gaps reset the HAM clock gate (1.2 vs 2.4 GHz).
  * The T^2-sized exp splits across TWO engines working in parallel:
      - ACT pairs: true exp (scale=2r) -> bf16.
      - DVE pairs: Schraudolph bit-trick exp: i16 = int16(G*(2r*c1)+c2)
        reinterpreted as bf16 IS approximately exp(2r*G) (~1.5% rms);
        one DVE tensor_scalar per pair.
    Pairs containing diagonal blocks (largest K values) are forced to
    ACT (exact exp).
  * mm2 (M=64) also runs as CONCURRENT dual col-tile pairs: s-block 2j
    accumulates into partitions 0:64 of the PSUM bank (tile (0,0)),
    s-block 2j+1 into partitions 64:128 (tile (0,64)).  Each t-block's
    [128, TB] accumulator is copied out in ONE op (halves stay in their
    partition ranges, ACT/DVE alternating per t-block).
  * Epilogue runs per HALF-batch: each [64, 1024] slice pair gets one
    DMA-xbar transpose per partition half, then big elementwise ops with
    stride-0 broadcast APs apply e_t and the +x residual (muls on DVE,
    adds on GpSimd).  Per-instruction overhead on DVE is ~200-400ns, so
    everything stays in >=512-row ops.
  * Batch b+1's prologue is EMITTED before batch b's epilogue so the
    Sync queue's head-of-line waits don't delay the next batch's x^T
    transpose chain.

Sharding: pure data-parallel over batch B=32 -> 4 batches per core x 8 cores.
"""

import os
import sys

import numpy as np

sys.path.insert(0, "/opt/trn_rl_repo")

import concourse.bass as bass
import concourse.tile as tile
from concourse import bacc, mybir
from concourse.bass_utils import run_bass_kernel_spmd

FP32 = mybir.dt.float32
BF16 = mybir.dt.bfloat16
I16 = mybir.dt.int16

B, T, C = 32, 2048, 64
N_CORES = 8
BPC = B // N_CORES  # batches per core

TB = 512            # t-block width (one PSUM bank of mm2 accumulation)

# Schraudolph exp-as-bf16-bits constants:  bf16_bits(z*SCHRAU_C1 + SCHRAU_C2)
# ~= exp(z).  c1 = 2^7/ln2; c2 = 127*2^7 - 7.42 (minimax shift) + 0.5
# (float->int truncation in the convert).
SCHRAU_C1 = 128.0 / 0.6931471805599453
SCHRAU_C2 = 16256.0 - 7.42 + 0.5

# Fraction of non-diagonal pairs whose exp runs on ACT (rest on DVE).
# DVE also carries prologue/epilogue elementwise work, so ACT takes a
# bigger share; diagonal pairs are forced to ACT on top of this.
ACT_NONDIAG_FRAC = 0.46

# Stashed by kernel() for the test harness (exec time etc.)
LAST_RESULTS = None


def _body(ctx, tc, out_ap, x_ap, xb_ap, r, bpc, t, dbg=False):
    """Emit the per-core kernel IR.

    out_ap/x_ap: DRAM APs of shape [bpc, t, C].
    xb_ap: DRAM AP [bpc, t, 2C] bf16 = [x | x] duplicated along channels
    (host-side cast+replication), transposed on-device into the dual
    row-group x^T layout.
    r: python float (r_sigma value, baked as immediates).
    """
    nc = tc.nc

    def dump(name, sb_ap, dt=None):
        if not dbg:
            return
        d = nc.dram_tensor(
            name, list(sb_ap.shape), dt or sb_ap.dtype, kind="ExternalOutput"
        ).ap()
        nc.sync.dma_start(out=d, in_=sb_ap)

    nt = t // 128          # 128-row s/t blocks
    ntb = t // TB
    npair = nt // 2
    nth = nt // 2          # 128-blocks per half-batch epilogue slice

    exp2r = 2.0 * r

    # SBUF pools.  Per-batch inputs/stats (x32, sq/ev, yb, xt) are bufs=4:
    # ALL batches' prologues run up front, so no prologue op ever sits in
    # an engine queue mid-run waiting on a DMA (head-of-line stalls there
    # starve the exp stream and re-throttle the PE clock gate).
    xpool = ctx.enter_context(tc.tile_pool(name="x32", bufs=bpc))
    xxpool = ctx.enter_context(tc.tile_pool(name="xx", bufs=2))
    sqpool = ctx.enter_context(tc.tile_pool(name="sq", bufs=bpc))
    ypool = ctx.enter_context(tc.tile_pool(name="yb", bufs=bpc))
    xtpool = ctx.enter_context(tc.tile_pool(name="xt", bufs=bpc))
    apool = ctx.enter_context(tc.tile_pool(name="a0", bufs=6))
    ipool = ctx.enter_context(tc.tile_pool(name="i16", bufs=6))
    otpool = ctx.enter_context(tc.tile_pool(name="otb", bufs=2))
    trpool = ctx.enter_context(tc.tile_pool(name="trb", bufs=6))
    opool = ctx.enter_context(tc.tile_pool(name="osb", bufs=6))
    # PSUM (8 banks total): g2 = [128, 2, TB] fp32 (2 banks) x3 bufs for the
    # two-step mm1 lookahead; p = [128, TB] (1 bank) x2 bufs
    gpool = ctx.enter_context(tc.tile_pool(name="gps", bufs=3, space="PSUM"))
    ppool = ctx.enter_context(tc.tile_pool(name="pps", bufs=2, space="PSUM"))

    state = {"act_credit": 0.0}
    batch = [None] * bpc   # per-batch tile dict

    def prologue(b):
        """Load, row stats, Y = e_s * x (bf16), duplicated X^T staging.

        The x^T staging is ONE DMA-xbar transpose straight from the
        duplicated bf16 DRAM input (xb = [x | x]) — no engine ops in the
        chain, so the matmul stream never waits on a queue-ordering hazard.
        xt[c, tt] = xt[64+c, tt] = x[tt, c] for c < 64.
        """
        xt = xtpool.tile([128, t], BF16)
        nc.sync.dma_start_transpose(out=xt[:], in_=xb_ap[b])

        xb_dram = x_ap[b].rearrange("(k p) c -> p k c", p=128)   # [128, nt, C]
        x32 = xpool.tile([128, nt, C], FP32)
        nc.sync.dma_start(out=x32[:], in_=xb_dram)

        # stats for the first two batches on DVE (needed soon); later
        # batches on the slower-but-idle GpSimd so the DVE queue stays
        # clear for the exp stream
        eng = nc.vector if b < 2 else nc.gpsimd
        xx = xxpool.tile([128, nt, C], FP32)
        eng.tensor_mul(xx[:], x32[:], x32[:])
        sq = sqpool.tile([128, nt], FP32, tag="sq")
        nc.vector.tensor_reduce(
            sq[:], xx[:], axis=mybir.AxisListType.X, op=mybir.AluOpType.add
        )
        ev = sqpool.tile([128, nt], FP32, tag="ev")
        nc.scalar.activation(
            ev[:], sq[:], mybir.ActivationFunctionType.Exp, scale=-r
        )
        ev_bc = ev[:, :, None].broadcast_to([128, nt, C])

        yb = ypool.tile([128, nt, C], BF16)
        eng.tensor_mul(yb[:], x32[:], ev_bc)

        if dbg and b == 0:
            dump("dbg_sq", sq[:])
            dump("dbg_ev", ev[:])
            dump("dbg_yb", yb[:])
            dump("dbg_xt", xt[:])
        batch[b] = dict(x32=x32, ev=ev, ev_bc=ev_bc, yb=yb, xt=xt)

    def main(b, mid_calls=None):
        """All mm1/exp/mm2 steps for one batch, mm1 two steps ahead.

        mid_calls: {step: callable} emitted at the given steps, so other
        batches' prologue/epilogue work lands at controlled positions in
        the per-engine queues (a dependency-blocked op at a queue head
        stalls everything behind it — the exp stream pacing the PE, or
        the x^T staging chain the next batch's matmuls wait on).
        """
        bt = batch[b]
        xt, yb = bt["xt"], bt["yb"]
        # otb partitions 0:64 hold the s-even half of out^T; partitions
        # 64:128 the s-odd half.
        otb = otpool.tile([128, t], BF16)
        bt["otb"] = otb

        steps = [(ti, j) for ti in range(ntb) for j in range(npair)]

        owners = []
        for ti, j in steps:
            if j in (2 * ti, 2 * ti + 1):
                owners.append("act")       # diagonal pair: exact exp
            else:
                state["act_credit"] += ACT_NONDIAG_FRAC
                if state["act_credit"] >= 1.0:
                    state["act_credit"] -= 1.0
                    owners.append("act")
                else:
                    owners.append("dve")

        def mm1(step):
            """Concurrent dual row-tile pair: G for s-blocks 2j, 2j+1."""
            ti, j = steps[step]
            g2 = gpool.tile([128, 2, TB], FP32, name="g_ps", tag="g")
            for i in range(2):
                base = 64 * i
                s = 2 * j + i
                nc.tensor.matmul(
                    g2[:, i],
                    lhsT=xt[base : base + 64, s * 128 : (s + 1) * 128],
                    rhs=xt[base : base + 64, ti * TB : (ti + 1) * TB],
                    start=True,
                    stop=True,
                )
            return g2

        p_ps = None
        gq = [mm1(0), mm1(1)]  # two-step lookahead queue
        for step, (ti, j) in enumerate(steps):
            pos = step % npair     # position within this t-block
            if mid_calls and step in mid_calls:
                mid_calls[step]()
            if pos == 0:
                p_ps = ppool.tile([128, TB], FP32, tag="p")
            g_cur = gq.pop(0)
            if step + 2 < len(steps):
                gq.append(mm1(step + 2))

            if owners[step] == "act":
                a0t = apool.tile([128, 2, TB], BF16)
                nc.scalar.activation(
                    a0t[:], g_cur[:], mybir.ActivationFunctionType.Exp,
                    scale=exp2r,
                )
                a0 = a0t[:]
            else:
                i16 = ipool.tile([128, 2, TB], I16)
                nc.vector.tensor_scalar(
                    i16[:],
                    g_cur[:],
                    exp2r * SCHRAU_C1,
                    SCHRAU_C2,
                    op0=mybir.AluOpType.mult,
                    op1=mybir.AluOpType.add,
                )
                a0 = i16[:].bitcast(BF16)
            if dbg and b == 0 and step == 0:
                gsb = xxpool.tile([128, 2, TB], FP32, tag="gdump")
                nc.vector.tensor_copy(gsb[:], g_cur[:])
                dump("dbg_g00", gsb[:])

            # mm2: concurrent dual col-tile pair -> partition halves of p_ps
            for i in range(2):
                nc.tensor.matmul(
                    p_ps[64 * i : 64 * i + 64, :],
                    lhsT=yb[:, 2 * j + i],
                    rhs=a0[:, i],
                    start=(pos == 0),
                    stop=(pos == npair - 1),
                    tile_position=(0, 64 * i),
                    skip_group_check=True,
                )

            if pos == npair - 1:
                # single full-width copy; halves stay in their partition
                # ranges.  Alternate ACT/DVE per t-block; the last batch's
                # final t-block goes to ACT (idle at the kernel tail).
                dst = otb[:, ti * TB : (ti + 1) * TB]
                on_act = ti % 2 == 0 or (b == bpc - 1 and ti == ntb - 1)
                if on_act:
                    nc.scalar.activation(
                        dst, p_ps[:], mybir.ActivationFunctionType.Copy
                    )
                else:
                    nc.vector.tensor_copy(dst, p_ps[:])

    def epilogue_half(b, h, nchunk=1, dve=False, act_store=False):
        """Transpose both out^T partition halves of one half-batch t-range,
        apply e_t scale and +x residual with big broadcast ops, store.

        Elementwise work goes to GpSimd by default: it has idle capacity,
        and a transpose-blocked op at the head of the ACT/DVE queues would
        stall the exp stream (and re-throttle the PE clock gate).  The
        LAST chunks run on DVE in small pieces instead (dve=True) —
        nothing else runs at the kernel tail and DVE is ~2x faster."""
        bt = batch[b]
        x32, ev, otb = bt["x32"], bt["ev"], bt["otb"]
        ob_dram = out_ap[b].rearrange("(k p) c -> p k c", p=128)
        nk = nth // nchunk           # 128-blocks per chunk
        eng = nc.vector if dve else nc.gpsimd
        trbs = []
        # all transposes first: a chunk's transpose must not queue behind
        # the previous chunk's elementwise chain.  ONE full-width [128, .]
        # transpose per chunk: the transposed s-even half lands in columns
        # 0:64, the s-odd half in 64:128.
        for q in range(nchunk):
            k0 = h * nth + q * nk
            tsl = slice(k0 * 128, (k0 + nk) * 128)
            trb = trpool.tile([128, nk, 2 * C], BF16, tag=f"trb{h}{q}")
            nc.sync.dma_start_transpose(out=trb[:], in_=otb[:, tsl])
            trbs.append(trb)
            if dbg and b == 0 and h == 0 and q == 0:
                dump("dbg_otb", otb[:])
                dump("dbg_trb", trb[:])
        for q in range(nchunk):
            k0 = h * nth + q * nk
            ksl = slice(k0, k0 + nk)
            trb = trbs[q]
            evh_bc = ev[:, ksl, None].broadcast_to([128, nk, C])
            o1 = opool.tile([128, nk, C], FP32, tag=f"o1{h}{q}")
            o2 = opool.tile([128, nk, C], FP32, tag=f"o2{h}{q}")
            osb = opool.tile([128, nk, C], FP32, tag=f"osb{h}{q}")
            eng.tensor_add(o1[:], trb[:, :, 0:C], trb[:, :, C : 2 * C])
            eng.tensor_mul(o2[:], o1[:], evh_bc)
            eng.tensor_add(osb[:], o2[:], x32[:, ksl])
            # near-tail chunks store via the ACT hwdge queue (idle then) so
            # the store's chain-wait never delays a later Sync transpose
            (nc.scalar if (dve or act_store) else nc.sync).dma_start(
                out=ob_dram[:, ksl], in_=osb[:]
            )

    for b in range(bpc):
        prologue(b)
    for b in range(bpc):
        mid = {
            16: lambda bb=b: epilogue_half(bb, 0, act_store=(bb == bpc - 1))
        }
        if b > 0:
            mid[6] = lambda bb=b - 1: epilogue_half(bb, 1)
        main(b, mid_calls=mid)
    # kernel tail: last half-batch in small DVE chunks
    epilogue_half(bpc - 1, 1, nchunk=2, dve=True)


def build(r, bpc=BPC, t=T, dbg=False):
    """Build + compile the Bass module for one core's shard."""
    from contextlib import ExitStack

    nc = bacc.Bacc(
        "TRN2", target_bir_lowering=False, debug=False, num_devices=N_CORES
    )
    x_ap = nc.dram_tensor("x", [bpc, t, C], FP32, kind="ExternalInput").ap()
    xb_ap = nc.dram_tensor(
        "xb", [bpc, t, 2 * C], BF16, kind="ExternalInput"
    ).ap()
    out_ap = nc.dram_tensor("out", [bpc, t, C], FP32, kind="ExternalOutput").ap()
    with tile.TileContext(nc) as tc:
        with ExitStack() as ctx:
            _body(ctx, tc, out_ap, x_ap, xb_ap, r, bpc, t, dbg=dbg)
    nc.compile()
    return nc


def kernel(x, r_sigma):
    global LAST_RESULTS
    x = np.ascontiguousarray(np.asarray(x, dtype=np.float32))
    r = float(np.asarray(r_sigma).reshape(-1)[0])
    assert x.shape == (B, T, C), x.shape

    import ml_dtypes

    nc = build(r)
    # duplicated bf16 copy of x ([x | x] along channels): pure host-side
    # formatting so the on-device x^T staging is a single DMA transpose
    xb = np.concatenate([x, x], axis=-1).astype(ml_dtypes.bfloat16)
    in_maps = [
        {
            "x": np.ascontiguousarray(x[i * BPC : (i + 1) * BPC]),
            "xb": np.ascontiguousarray(xb[i * BPC : (i + 1) * BPC]),
        }
        for i in range(N_CORES)
    ]
    trace = bool(int(os.environ.get("KERNEL_TRACE", "0")))
    res = run_bass_kernel_spmd(
        nc, in_maps, core_ids=list(range(N_CORES)), trace=trace
    )
    LAST_RESULTS = res
    out = np.concatenate([res.results[i]["out"] for i in range(N_CORES)], axis=0)
    return out.astype(np.float32)
